# revision 10
# baseline (speedup 1.0000x reference)
"""Trainium2 Bass kernel for the QA-head top-k span-masking problem.

Computation (per batch b):
    logits = seq_hiddens[b] @ W_qa + b_qa          # (S, 2)
    masked = logits * m + (1 - m) * (-1e30)        # ans_mask
    start, end = masked[:, 0], masked[:, 1]
    span[i, j] = start[i] + end[j]  valid iff (i >= 4 and 0 <= j - i < 30)
                                     or (i == j in {1, 2, 3})
    top-5 spans by score (descending), flat index i * S + j

Sharding: pure data parallel, batch b -> NeuronCore b (B == 8 == n_cores).

seq_hiddens is staged pre-transposed and split into a bf16 hi/lo pair
(x = xh + xl, W staged as [Wh | 0 | Wl]): fp32 matmuls run at 4
cycles/row on the PE (above the DMA roofline) and float32r corrupts the
DVE max8 path on this toolchain, while bf16 runs at 1 cycle/row and the
four bf16 x bf16 cross products are exact in the fp32 PSUM accumulator,
so the split matmul matches fp32 to ~1e-6. HBM traffic is unchanged
(2 x 8.4 MB bf16 per core = the same 16.8 MB stream, the memory
roofline). The transpose is staged on the host because a 4-byte
transposed DMA degenerates to 4-byte descriptors.

Per core: the sync HWDGE ring streams x^T once (big DMAs only; the
small constant/reshape/output DMAs ride the scalar HWDGE ring so they
never stall the stream); 16 accumulating [128, 34] x [128, 512] bf16
matmuls per 512-column block -> PSUM rows 0-1 (x @ Wh) and 32-33
(x @ Wl, placed at 32 because PSUM reads must start at partition
0/32/64/96); ScalarE copies the Wl half to SBUF, VectorE folds, adds
bias, applies ans_mask into flat [2, 4096] logits; per-block DMAs
reshape start/end into a [128, 32] layout (i = 32p + f) while the
stream continues; VectorE builds the 30 shifted-diagonal candidate
bands C[p, 32d + f] = s[i] + e[i + d] in four 32-partition groups (each
scheduled as soon as its logits land) and runs the hardware
per-partition top-8 (max / max_index). The host reduces the 128x8
per-partition maxima to the global top-5, re-scoring the 1024 candidate
spans in exact fp32 as tie-safety.
"""

import numpy as np

try:
    import concourse.bass as bass  # noqa: F401
except ImportError:  # pragma: no cover - container staging path
    import sys

    sys.path.insert(0, "/opt/trn_rl_repo")

import concourse.bass as bass
import concourse.tile as tile
from concourse import bacc, mybir
from concourse.bass_utils import run_bass_kernel_spmd

B, S, H = 8, 4096, 1024
N_CORES = 8
SBLK = 512              # s-range per PSUM accumulation block
NSB = S // SBLK         # 8 s-blocks
PPB = SBLK // 32        # partition rows of the [128, 32] layout per block (16)
KC = H // 128           # 8 contraction chunks
MAXLEN = 30             # spans have 0 <= j - i < 30
NEG = -1.0e30
F32 = mybir.dt.float32
BF16 = mybir.dt.bfloat16
U32 = mybir.dt.uint32

_CACHE = {}


def _build():
    nc = bacc.Bacc("TRN2", target_bir_lowering=False, debug=False,
                   num_devices=N_CORES)
    xh = nc.dram_tensor("xh", [H, S], BF16, kind="ExternalInput").ap()
    xl = nc.dram_tensor("xl", [H, S], BF16, kind="ExternalInput").ap()
    whl = nc.dram_tensor("whl", [H, 34], BF16, kind="ExternalInput").ap()
    bq = nc.dram_tensor("bq", [2, 1], F32, kind="ExternalInput").ap()
    am = nc.dram_tensor("am", [2, S], F32, kind="ExternalInput").ap()
    out_logits = nc.dram_tensor("out_logits", [2, S], F32,
                                kind="ExternalOutput").ap()
    out_m8 = nc.dram_tensor("out_m8", [128, 8], F32, kind="ExternalOutput").ap()
    out_i8 = nc.dram_tensor("out_i8", [128, 8], U32, kind="ExternalOutput").ap()

    with tile.TileContext(nc) as tc:
        with (
            tc.tile_pool(name="const", bufs=1) as cpool,
            tc.tile_pool(name="xin", bufs=4) as xpool,
            tc.tile_pool(name="blk", bufs=2) as bpool,
            tc.tile_pool(name="psum", bufs=2, space="PSUM") as ppool,
            tc.tile_pool(name="work", bufs=1) as wpool,
        ):
            # stationary layout [Wh(2) | zeros(30) | Wl(2)] per K-chunk, all
            # 8 chunks loaded in one DMA on the scalar ring
            w_sb = cpool.tile([128, KC, 34], BF16)
            nc.scalar.dma_start(w_sb[:],
                                whl.rearrange("(k p) c -> p k c", p=128))
            b_sb = cpool.tile([2, 1], F32)
            nc.scalar.dma_start(b_sb[:], bq[:])
            am_sb = cpool.tile([2, S], F32)
            nc.scalar.dma_start(am_sb[:], am[:])

            logits_sb = wpool.tile([2, S], F32)
            s4 = wpool.tile([128, 32], F32)
            e_ext = wpool.tile([128, 64], F32)
            nc.vector.memset(e_ext[96:128, 32:32 + MAXLEN], NEG)

            for sb in range(NSB):
                xh_t = xpool.tile([128, KC, SBLK], BF16, tag="xh")
                xl_t = xpool.tile([128, KC, SBLK], BF16, tag="xl")
                seg = slice(SBLK * sb, SBLK * (sb + 1))
                nc.sync.dma_start(
                    xh_t[:], xh[:, seg].rearrange("(k p) s -> p k s", p=128))
                nc.sync.dma_start(
                    xl_t[:], xl[:, seg].rearrange("(k p) s -> p k s", p=128))
                # PSUM rows 0-1: x? @ Wh, rows 32-33: x? @ Wl; accumulating
                # both the xh and xl passes gives the exact 4-term product.
                pt = ppool.tile([34, SBLK], F32)
                for kc in range(KC):
                    nc.tensor.matmul(pt[:], w_sb[:, kc, :], xh_t[:, kc, :],
                                     start=(kc == 0), stop=False)
                for kc in range(KC):
                    nc.tensor.matmul(pt[:], w_sb[:, kc, :], xl_t[:, kc, :],
                                     start=False, stop=(kc == KC - 1))
                # TensorTensor may read only one PSUM operand: stage the Wl
                # half through SBUF via an exact ScalarE copy first.
                t_lo = bpool.tile([2, SBLK], F32, tag="tlo")
                nc.scalar.copy(t_lo[:], pt[32:34, :])
                t_hl = bpool.tile([2, SBLK], F32, tag="thl")
                nc.vector.tensor_add(t_hl[:], pt[0:2, :], t_lo[:])
                t_bias = bpool.tile([2, SBLK], F32, tag="tbias")
                nc.vector.tensor_scalar(t_bias[:], t_hl[:], b_sb[:, 0:1], None,
                                        mybir.AluOpType.add)
                # masked = (x+b)*m + (m*1e30 - 1e30)
                t_nm = bpool.tile([2, SBLK], F32, tag="tnm")
                nc.vector.tensor_scalar(t_nm[:], am_sb[:, seg], 1.0e30, NEG,
                                        mybir.AluOpType.mult,
                                        mybir.AluOpType.add)
                t_p = bpool.tile([2, SBLK], F32, tag="tp")
                nc.vector.tensor_mul(t_p[:], t_bias[:], am_sb[:, seg])
                nc.vector.tensor_add(logits_sb[:, seg], t_p[:], t_nm[:])

                # Reshape this block's start/end rows into the [128, 32]
                # (i = 32p + f) layout while later blocks still stream.
                prow = slice(PPB * sb, PPB * (sb + 1))
                nc.scalar.dma_start(s4[prow, :], logits_sb[0:1, seg])
                nc.scalar.dma_start(e_ext[prow, 0:32], logits_sb[1:2, seg])
                nc.scalar.dma_start(out_logits[:, seg], logits_sb[:, seg])
                if sb == 0:
                    # start positions 0..3 are invalid for every d >= 1 and
                    # for (0,0); specials are re-added below
                    nc.vector.memset(s4[0:1, 0:4], NEG)
                # e_ext[p, 32+t] = e[32(p+1) + t] (next-partition spill) for
                # the previous block's rows: its sources end 30 elements into
                # this block. Rows 96..126 resolve within the last block; row
                # 127 keeps the NEG memset so spans with j >= S stay invalid.
                def spill(p0, nrows):
                    lo = 32 * p0 + 32
                    src = logits_sb[1:2, lo:lo + 32 * nrows].rearrange(
                        "a (p t) -> a p t", t=32)[:, :, 0:MAXLEN]
                    nc.scalar.dma_start(e_ext[p0:p0 + nrows, 32:32 + MAXLEN],
                                        src)
                if sb > 0:
                    spill(PPB * (sb - 1), PPB)
                if sb == NSB - 1:
                    spill(PPB * sb, PPB - 1)

            cand = wpool.tile([128, 32 * MAXLEN], F32)
            for g in range(4):
                gp = slice(32 * g, 32 * (g + 1))
                for d in range(MAXLEN):
                    nc.vector.tensor_add(cand[gp, 32 * d:32 * d + 32],
                                         s4[gp, :], e_ext[gp, d:d + 32])
            # special diagonal cells (1,1), (2,2), (3,3) are valid at d = 0
            nc.vector.tensor_add(cand[0:1, 1:4], logits_sb[0:1, 1:4],
                                 e_ext[0:1, 1:4])

            m8 = wpool.tile([128, 8], F32)
            i8 = wpool.tile([128, 8], U32)
            nc.vector.max(m8[:], cand[:])
            nc.vector.max_index(i8[:], m8[:], cand[:])
            nc.scalar.dma_start(out_m8[:], m8[:])
            nc.scalar.dma_start(out_i8[:], i8[:])

    nc.compile()
    return nc


def _get_nc():
    if "nc" not in _CACHE:
        _CACHE["nc"] = _build()
    return _CACHE["nc"]


def _split_bf16(a):
    """a (f32) -> (hi, lo) bf16 with hi + lo ~= a."""
    import ml_dtypes
    hi = a.astype(ml_dtypes.bfloat16)
    lo = (a - hi.astype(np.float32)).astype(ml_dtypes.bfloat16)
    return hi, lo


def run_device(seq_hiddens, ans_mask, W_qa, b_qa, trace=False, **kw):
    nc = _get_nc()
    seq_hiddens = np.asarray(seq_hiddens, dtype=np.float32)
    ans_mask = np.asarray(ans_mask, dtype=np.float32)
    w = np.asarray(W_qa, dtype=np.float32)
    wh, wl = _split_bf16(w)
    import ml_dtypes
    whl = np.zeros((H, 34), ml_dtypes.bfloat16)
    whl[:, 0:2] = wh
    whl[:, 32:34] = wl
    whl = np.ascontiguousarray(whl)
    bq = np.ascontiguousarray(np.asarray(b_qa, dtype=np.float32).reshape(2, 1))
    in_maps = []
    for b in range(N_CORES):
        xt = np.ascontiguousarray(seq_hiddens[b].T)
        xhb, xlb = _split_bf16(xt)
        in_maps.append({
            "xh": np.ascontiguousarray(xhb),
            "xl": np.ascontiguousarray(xlb),
            "whl": whl,
            "bq": bq,
            "am": np.ascontiguousarray(
                np.broadcast_to(ans_mask[b][None, :], (2, S))),
        })
    return run_bass_kernel_spmd(nc, in_maps, core_ids=list(range(N_CORES)),
                                trace=trace, **kw)


def kernel(seq_hiddens, ans_mask, W_qa, b_qa, top_k):
    k = int(top_k)
    assert k <= 8
    seq_hiddens = np.asarray(seq_hiddens, dtype=np.float32)
    ans_mask = np.asarray(ans_mask, dtype=np.float32)
    w = np.asarray(W_qa, dtype=np.float32)
    bq = np.asarray(b_qa, dtype=np.float32).reshape(2)
    res = run_device(seq_hiddens, ans_mask, w, bq)
    start_logits = np.empty((B, S), np.float32)
    end_logits = np.empty((B, S), np.float32)
    top_start = np.empty((B, k), np.int32)
    top_end = np.empty((B, k), np.int32)
    for b in range(B):
        out = res.results[b]
        start_logits[b] = out["out_logits"][0]
        end_logits[b] = out["out_logits"][1]
        # Decode the 1024 device-selected candidate spans, then re-score
        # them in exact fp32 as insurance against near-ties.
        q = out["out_i8"].astype(np.int64).ravel()            # [1024]
        p = np.arange(128, dtype=np.int64).repeat(8)
        d, f = q // 32, q % 32
        ii = 32 * p + f
        jj = ii + d
        x = seq_hiddens[b]
        m = ans_mask[b]
        s_exact = (x[ii] @ w[:, 0] + bq[0]) * m[ii] + (1.0 - m[ii]) * NEG
        e_exact = (x[jj] @ w[:, 1] + bq[1]) * m[jj] + (1.0 - m[jj]) * NEG
        score = s_exact.astype(np.float64) + e_exact.astype(np.float64)
        flat = ii * S + jj
        order = np.lexsort((flat, -score))[:k]
        top_start[b] = ii[order].astype(np.int32)
        top_end[b] = jj[order].astype(np.int32)
    return start_logits, end_logits, top_start, top_end


# revision 11
# speedup vs baseline: 1.1706x; 1.1706x over previous
"""Trainium2 Bass kernel for the QA-head top-k span-masking problem.

Computation (per batch b):
    logits = seq_hiddens[b] @ W_qa + b_qa          # (S, 2)
    masked = logits * m + (1 - m) * (-1e30)        # ans_mask
    start, end = masked[:, 0], masked[:, 1]
    span[i, j] = start[i] + end[j]  valid iff (i >= 4 and 0 <= j - i < 30)
                                     or (i == j in {1, 2, 3})
    top-5 spans by score (descending), flat index i * S + j

Sharding: pure data parallel, batch b -> NeuronCore b (B == 8 == n_cores).

seq_hiddens is staged pre-transposed and split into a bf16 hi/lo pair
(x = xh + xl, W staged as [Wh | 0 | Wl]): fp32 matmuls run at 4
cycles/row on the PE (above the DMA roofline) and float32r corrupts the
DVE max8 path on this toolchain, while bf16 runs at 1 cycle/row and the
four bf16 x bf16 cross products are exact in the fp32 PSUM accumulator,
so the split matmul matches fp32 to ~1e-6. HBM traffic is unchanged
(2 x 8.4 MB bf16 per core = the same 16.8 MB stream, the memory
roofline). The transpose is staged on the host because a 4-byte
transposed DMA degenerates to 4-byte descriptors.

Per core: the sync HWDGE ring streams x^T once (big DMAs only; the
small constant/reshape/output DMAs ride the scalar HWDGE ring so they
never stall the stream); 16 accumulating [128, 34] x [128, 512] bf16
matmuls per 512-column block -> PSUM rows 0-1 (x @ Wh) and 32-33
(x @ Wl, placed at 32 because PSUM reads must start at partition
0/32/64/96); ScalarE copies the Wl half to SBUF, VectorE folds, adds
bias, applies ans_mask into flat [2, 4096] logits; per-block DMAs
reshape start/end into a [128, 32] layout (i = 32p + f) while the
stream continues; VectorE builds the 30 shifted-diagonal candidate
bands C[p, 32d + f] = s[i] + e[i + d] in four 32-partition groups (each
scheduled as soon as its logits land) and runs the hardware
per-partition top-8 (max / max_index). The host reduces the 128x8
per-partition maxima to the global top-5, re-scoring the 1024 candidate
spans in exact fp32 as tie-safety.
"""

import numpy as np

try:
    import concourse.bass as bass  # noqa: F401
except ImportError:  # pragma: no cover - container staging path
    import sys

    sys.path.insert(0, "/opt/trn_rl_repo")

import concourse.bass as bass
import concourse.tile as tile
from concourse import bacc, mybir
from concourse.bass_utils import run_bass_kernel_spmd

B, S, H = 8, 4096, 1024
N_CORES = 8
SBLK = 512              # s-range per PSUM accumulation block
NSB = S // SBLK         # 8 s-blocks
PPB = SBLK // 32        # partition rows of the [128, 32] layout per block (16)
KC = H // 128           # 8 contraction chunks
MAXLEN = 30             # spans have 0 <= j - i < 30
NEG = -1.0e30
F32 = mybir.dt.float32
BF16 = mybir.dt.bfloat16
U32 = mybir.dt.uint32

_CACHE = {}


def _build():
    nc = bacc.Bacc("TRN2", target_bir_lowering=False, debug=False,
                   num_devices=N_CORES)
    xh = nc.dram_tensor("xh", [H, S], BF16, kind="ExternalInput").ap()
    xl = nc.dram_tensor("xl", [H, S], BF16, kind="ExternalInput").ap()
    whl = nc.dram_tensor("whl", [H, 34], BF16, kind="ExternalInput").ap()
    bq = nc.dram_tensor("bq", [2, 1], F32, kind="ExternalInput").ap()
    am = nc.dram_tensor("am", [2, S], F32, kind="ExternalInput").ap()
    out_logits = nc.dram_tensor("out_logits", [2, S], F32,
                                kind="ExternalOutput").ap()
    out_m8 = nc.dram_tensor("out_m8", [128, 8], F32, kind="ExternalOutput").ap()
    out_i8 = nc.dram_tensor("out_i8", [128, 8], U32, kind="ExternalOutput").ap()

    with tile.TileContext(nc) as tc:
        with (
            tc.tile_pool(name="const", bufs=1) as cpool,
            tc.tile_pool(name="xin", bufs=6) as xpool,
            tc.tile_pool(name="blk", bufs=2) as bpool,
            tc.tile_pool(name="psum", bufs=3, space="PSUM") as ppool,
            tc.tile_pool(name="work", bufs=1) as wpool,
        ):
            # stationary layout [Wh(2) | zeros(30) | Wl(2)] per K-chunk, all
            # 8 chunks loaded in one DMA on the scalar ring
            w_sb = cpool.tile([128, KC, 34], BF16)
            nc.scalar.dma_start(w_sb[:],
                                whl.rearrange("(k p) c -> p k c", p=128))
            b_sb = cpool.tile([2, 1], F32)
            nc.scalar.dma_start(b_sb[:], bq[:])
            am_sb = cpool.tile([2, S], F32)
            nc.scalar.dma_start(am_sb[:], am[:])
            # nm = m*1e30 - 1e30 == (1-m)*(-1e30), hoisted out of the block loop
            nm_sb = cpool.tile([2, S], F32)
            nc.vector.tensor_scalar(nm_sb[:], am_sb[:], 1.0e30, NEG,
                                    mybir.AluOpType.mult,
                                    mybir.AluOpType.add)

            logits_sb = wpool.tile([2, S], F32)
            s4 = wpool.tile([128, 32], F32)
            e_ext = wpool.tile([128, 64], F32)
            nc.vector.memset(e_ext[96:128, 32:32 + MAXLEN], NEG)

            for sb in range(NSB):
                xh_t = xpool.tile([128, KC, SBLK], BF16, tag="xh")
                xl_t = xpool.tile([128, KC, SBLK], BF16, tag="xl")
                seg = slice(SBLK * sb, SBLK * (sb + 1))
                nc.sync.dma_start(
                    xh_t[:], xh[:, seg].rearrange("(k p) s -> p k s", p=128))
                nc.sync.dma_start(
                    xl_t[:], xl[:, seg].rearrange("(k p) s -> p k s", p=128))
                # PSUM rows 0-1: x? @ Wh, rows 32-33: x? @ Wl; accumulating
                # both the xh and xl passes gives the exact 4-term product.
                pt = ppool.tile([34, SBLK], F32)
                for kc in range(KC):
                    nc.tensor.matmul(pt[:], w_sb[:, kc, :], xh_t[:, kc, :],
                                     start=(kc == 0), stop=False)
                for kc in range(KC):
                    nc.tensor.matmul(pt[:], w_sb[:, kc, :], xl_t[:, kc, :],
                                     start=False, stop=(kc == KC - 1))
                # fold: (pt_hi + b) + pt_lo, each op reading one PSUM operand
                t_bias = bpool.tile([2, SBLK], F32, tag="tbias")
                nc.vector.tensor_scalar(t_bias[:], pt[0:2, :], b_sb[:, 0:1],
                                        None, mybir.AluOpType.add)
                t_hl = bpool.tile([2, SBLK], F32, tag="thl")
                nc.vector.tensor_add(t_hl[:], pt[32:34, :], t_bias[:])
                # masked = (x+b)*m + nm
                t_p = bpool.tile([2, SBLK], F32, tag="tp")
                nc.vector.tensor_mul(t_p[:], t_hl[:], am_sb[:, seg])
                nc.vector.tensor_add(logits_sb[:, seg], t_p[:], nm_sb[:, seg])

                # Reshape this block's start/end rows into the [128, 32]
                # (i = 32p + f) layout while later blocks still stream.
                prow = slice(PPB * sb, PPB * (sb + 1))
                nc.scalar.dma_start(s4[prow, :], logits_sb[0:1, seg])
                nc.scalar.dma_start(e_ext[prow, 0:32], logits_sb[1:2, seg])
                nc.scalar.dma_start(out_logits[:, seg], logits_sb[:, seg])
                if sb == 0:
                    # start positions 0..3 are invalid for every d >= 1 and
                    # for (0,0); specials are re-added below
                    nc.vector.memset(s4[0:1, 0:4], NEG)
                # e_ext[p, 32+t] = e[32(p+1) + t] (next-partition spill) for
                # the previous block's rows: its sources end 30 elements into
                # this block. Rows 96..126 resolve within the last block; row
                # 127 keeps the NEG memset so spans with j >= S stay invalid.
                def spill(p0, nrows):
                    lo = 32 * p0 + 32
                    src = logits_sb[1:2, lo:lo + 32 * nrows].rearrange(
                        "a (p t) -> a p t", t=32)[:, :, 0:MAXLEN]
                    nc.scalar.dma_start(e_ext[p0:p0 + nrows, 32:32 + MAXLEN],
                                        src)
                if sb > 0:
                    spill(PPB * (sb - 1), PPB)
                if sb == NSB - 1:
                    spill(PPB * sb, PPB - 1)

            cand = wpool.tile([128, 32 * MAXLEN], F32)
            for d in range(MAXLEN):
                nc.vector.tensor_add(cand[:, 32 * d:32 * d + 32],
                                     s4[:], e_ext[:, d:d + 32])
            # special diagonal cells (1,1), (2,2), (3,3) are valid at d = 0
            nc.vector.tensor_add(cand[0:1, 1:4], logits_sb[0:1, 1:4],
                                 e_ext[0:1, 1:4])

            m8 = wpool.tile([128, 8], F32)
            i8 = wpool.tile([128, 8], U32)
            nc.vector.max(m8[:], cand[:])
            nc.vector.max_index(i8[:], m8[:], cand[:])
            nc.scalar.dma_start(out_m8[:], m8[:])
            nc.scalar.dma_start(out_i8[:], i8[:])

    nc.compile()
    return nc


def _get_nc():
    if "nc" not in _CACHE:
        _CACHE["nc"] = _build()
    return _CACHE["nc"]


def _split_bf16(a):
    """a (f32) -> (hi, lo) bf16 with hi + lo ~= a."""
    import ml_dtypes
    hi = a.astype(ml_dtypes.bfloat16)
    lo = (a - hi.astype(np.float32)).astype(ml_dtypes.bfloat16)
    return hi, lo


def run_device(seq_hiddens, ans_mask, W_qa, b_qa, trace=False, **kw):
    nc = _get_nc()
    seq_hiddens = np.asarray(seq_hiddens, dtype=np.float32)
    ans_mask = np.asarray(ans_mask, dtype=np.float32)
    w = np.asarray(W_qa, dtype=np.float32)
    wh, wl = _split_bf16(w)
    import ml_dtypes
    whl = np.zeros((H, 34), ml_dtypes.bfloat16)
    whl[:, 0:2] = wh
    whl[:, 32:34] = wl
    whl = np.ascontiguousarray(whl)
    bq = np.ascontiguousarray(np.asarray(b_qa, dtype=np.float32).reshape(2, 1))
    in_maps = []
    for b in range(N_CORES):
        xt = np.ascontiguousarray(seq_hiddens[b].T)
        xhb, xlb = _split_bf16(xt)
        in_maps.append({
            "xh": np.ascontiguousarray(xhb),
            "xl": np.ascontiguousarray(xlb),
            "whl": whl,
            "bq": bq,
            "am": np.ascontiguousarray(
                np.broadcast_to(ans_mask[b][None, :], (2, S))),
        })
    return run_bass_kernel_spmd(nc, in_maps, core_ids=list(range(N_CORES)),
                                trace=trace, **kw)


def kernel(seq_hiddens, ans_mask, W_qa, b_qa, top_k):
    k = int(top_k)
    assert k <= 8
    seq_hiddens = np.asarray(seq_hiddens, dtype=np.float32)
    ans_mask = np.asarray(ans_mask, dtype=np.float32)
    w = np.asarray(W_qa, dtype=np.float32)
    bq = np.asarray(b_qa, dtype=np.float32).reshape(2)
    res = run_device(seq_hiddens, ans_mask, w, bq)
    start_logits = np.empty((B, S), np.float32)
    end_logits = np.empty((B, S), np.float32)
    top_start = np.empty((B, k), np.int32)
    top_end = np.empty((B, k), np.int32)
    for b in range(B):
        out = res.results[b]
        start_logits[b] = out["out_logits"][0]
        end_logits[b] = out["out_logits"][1]
        # Decode the 1024 device-selected candidate spans, then re-score
        # them in exact fp32 as insurance against near-ties.
        q = out["out_i8"].astype(np.int64).ravel()            # [1024]
        p = np.arange(128, dtype=np.int64).repeat(8)
        d, f = q // 32, q % 32
        ii = 32 * p + f
        jj = ii + d
        x = seq_hiddens[b]
        m = ans_mask[b]
        s_exact = (x[ii] @ w[:, 0] + bq[0]) * m[ii] + (1.0 - m[ii]) * NEG
        e_exact = (x[jj] @ w[:, 1] + bq[1]) * m[jj] + (1.0 - m[jj]) * NEG
        score = s_exact.astype(np.float64) + e_exact.astype(np.float64)
        flat = ii * S + jj
        order = np.lexsort((flat, -score))[:k]
        top_start[b] = ii[order].astype(np.int32)
        top_end[b] = jj[order].astype(np.int32)
    return start_logits, end_logits, top_start, top_end


# revision 12
# speedup vs baseline: 1.2800x; 1.0934x over previous
"""Trainium2 Bass kernel for the QA-head top-k span-masking problem.

Computation (per batch b):
    logits = seq_hiddens[b] @ W_qa + b_qa          # (S, 2)
    masked = logits * m + (1 - m) * (-1e30)        # ans_mask
    start, end = masked[:, 0], masked[:, 1]
    span[i, j] = start[i] + end[j]  valid iff (i >= 4 and 0 <= j - i < 30)
                                     or (i == j in {1, 2, 3})
    top-5 spans by score (descending), flat index i * S + j

Sharding: pure data parallel, batch b -> NeuronCore b (B == 8 == n_cores).

seq_hiddens is staged pre-transposed and split into a bf16 hi/lo pair
(x = xh + xl, W staged as [Wh | 0 | Wl]): fp32 matmuls run at 4
cycles/row on the PE (above the DMA roofline) and float32r corrupts the
DVE max8 path on this toolchain, while bf16 runs at 1 cycle/row and the
four bf16 x bf16 cross products are exact in the fp32 PSUM accumulator,
so the split matmul matches fp32 to ~1e-6. HBM traffic is unchanged
(2 x 8.4 MB bf16 per core = the same 16.8 MB stream, the memory
roofline). The transpose is staged on the host because a 4-byte
transposed DMA degenerates to 4-byte descriptors.

Per core: the sync HWDGE ring streams x^T once (big DMAs only; the
small constant/reshape/output DMAs ride the scalar HWDGE ring so they
never stall the stream); 16 accumulating [128, 34] x [128, 512] bf16
matmuls per 512-column block -> PSUM rows 0-1 (x @ Wh) and 32-33
(x @ Wl, placed at 32 because PSUM reads must start at partition
0/32/64/96); ScalarE copies the Wl half to SBUF, VectorE folds, adds
bias, applies ans_mask into flat [2, 4096] logits; per-block DMAs
reshape start/end into a [128, 32] layout (i = 32p + f) while the
stream continues; VectorE builds the 30 shifted-diagonal candidate
bands C[p, 32d + f] = s[i] + e[i + d] in four 32-partition groups (each
scheduled as soon as its logits land) and runs the hardware
per-partition top-8 (max / max_index). The host reduces the 128x8
per-partition maxima to the global top-5, re-scoring the 1024 candidate
spans in exact fp32 as tie-safety.
"""

import numpy as np

try:
    import concourse.bass as bass  # noqa: F401
except ImportError:  # pragma: no cover - container staging path
    import sys

    sys.path.insert(0, "/opt/trn_rl_repo")

import concourse.bass as bass
import concourse.tile as tile
from concourse import bacc, mybir
from concourse.bass_utils import run_bass_kernel_spmd

B, S, H = 8, 4096, 1024
N_CORES = 8
SBLK = 512              # s-range per PSUM accumulation block
NSB = S // SBLK         # 8 s-blocks
PPB = SBLK // 32        # partition rows of the [128, 32] layout per block (16)
KC = H // 128           # 8 contraction chunks
MAXLEN = 30             # spans have 0 <= j - i < 30
NEG = -1.0e30
F32 = mybir.dt.float32
BF16 = mybir.dt.bfloat16
U32 = mybir.dt.uint32

_CACHE = {}


def _build():
    nc = bacc.Bacc("TRN2", target_bir_lowering=False, debug=False,
                   num_devices=N_CORES)
    xh = nc.dram_tensor("xh", [H, S], BF16, kind="ExternalInput").ap()
    xl = nc.dram_tensor("xl", [H, S], BF16, kind="ExternalInput").ap()
    whl = nc.dram_tensor("whl", [H, 34], BF16, kind="ExternalInput").ap()
    bq = nc.dram_tensor("bq", [2, 1], F32, kind="ExternalInput").ap()
    am = nc.dram_tensor("am", [2, S], F32, kind="ExternalInput").ap()
    out_logits = nc.dram_tensor("out_logits", [2, S], F32,
                                kind="ExternalOutput").ap()
    out_m8 = nc.dram_tensor("out_m8", [128, 8], F32, kind="ExternalOutput").ap()
    out_i8 = nc.dram_tensor("out_i8", [128, 8], U32, kind="ExternalOutput").ap()

    with tile.TileContext(nc) as tc:
        with (
            tc.tile_pool(name="const", bufs=1) as cpool,
            tc.tile_pool(name="xin", bufs=6) as xpool,
            tc.tile_pool(name="blk", bufs=2) as bpool,
            tc.tile_pool(name="psum", bufs=3, space="PSUM") as ppool,
            tc.tile_pool(name="work", bufs=1) as wpool,
        ):
            # stationary layout [Wh(2) | zeros(30) | Wl(2)] per K-chunk, all
            # 8 chunks loaded in one DMA on the scalar ring
            w_sb = cpool.tile([128, KC, 34], BF16)
            nc.scalar.dma_start(w_sb[:],
                                whl.rearrange("(k p) c -> p k c", p=128))
            b_sb = cpool.tile([2, 1], F32)
            nc.scalar.dma_start(b_sb[:], bq[:])
            am_sb = cpool.tile([2, S], F32)
            nc.scalar.dma_start(am_sb[:], am[:])
            # nm = m*1e30 - 1e30 == (1-m)*(-1e30), hoisted out of the block loop
            nm_sb = cpool.tile([2, S], F32)
            nc.vector.tensor_scalar(nm_sb[:], am_sb[:], 1.0e30, NEG,
                                    mybir.AluOpType.mult,
                                    mybir.AluOpType.add)

            logits_sb = wpool.tile([2, S], F32)
            s4 = wpool.tile([128, 32], F32)
            e_ext = wpool.tile([128, 64], F32)
            nc.vector.memset(e_ext[96:128, 32:32 + MAXLEN], NEG)

            sizes = [512] * 7 + [256, 256]
            starts = [sum(sizes[:i]) for i in range(len(sizes))]
            for sb, (s0, sz) in enumerate(zip(starts, sizes)):
                xh_t = xpool.tile([128, KC, sz], BF16, tag="xh")
                xl_t = xpool.tile([128, KC, sz], BF16, tag="xl")
                seg = slice(s0, s0 + sz)
                nc.sync.dma_start(
                    xh_t[:], xh[:, seg].rearrange("(k p) s -> p k s", p=128))
                nc.sync.dma_start(
                    xl_t[:], xl[:, seg].rearrange("(k p) s -> p k s", p=128))
                # PSUM rows 0-1: x? @ Wh, rows 32-33: x? @ Wl; accumulating
                # both the xh and xl passes gives the exact 4-term product.
                pt = ppool.tile([34, sz], F32, tag="pt")
                for kc in range(KC):
                    nc.tensor.matmul(pt[:], w_sb[:, kc, :], xh_t[:, kc, :],
                                     start=(kc == 0), stop=False)
                for kc in range(KC):
                    nc.tensor.matmul(pt[:], w_sb[:, kc, :], xl_t[:, kc, :],
                                     start=False, stop=(kc == KC - 1))
                # fold: (pt_hi + b) + pt_lo, each op reading one PSUM operand
                t_bias = bpool.tile([2, sz], F32, tag="tbias")
                nc.vector.tensor_scalar(t_bias[:], pt[0:2, :], b_sb[:, 0:1],
                                        None, mybir.AluOpType.add)
                t_hl = bpool.tile([2, sz], F32, tag="thl")
                nc.vector.tensor_add(t_hl[:], pt[32:34, :], t_bias[:])
                # masked = (x+b)*m + nm
                t_p = bpool.tile([2, sz], F32, tag="tp")
                nc.vector.tensor_mul(t_p[:], t_hl[:], am_sb[:, seg])
                nc.vector.tensor_add(logits_sb[:, seg], t_p[:], nm_sb[:, seg])

                # Reshape this block's start/end rows into the [128, 32]
                # (i = 32p + f) layout while later blocks still stream.
                prow = slice(s0 // 32, (s0 + sz) // 32)
                nc.scalar.dma_start(s4[prow, :], logits_sb[0:1, seg])
                nc.scalar.dma_start(e_ext[prow, 0:32], logits_sb[1:2, seg])
                nc.scalar.dma_start(out_logits[:, seg], logits_sb[:, seg])
                if sb == 0:
                    # start positions 0..3 are invalid for every d >= 1 and
                    # for (0,0); specials are re-added below
                    nc.vector.memset(s4[0:1, 0:4], NEG)
                # e_ext[p, 32+t] = e[32(p+1) + t] (next-partition spill) for
                # the previous block's rows: its sources end 30 elements into
                # this block. Rows 96..126 resolve within the last block; row
                # 127 keeps the NEG memset so spans with j >= S stay invalid.
                def spill(p0, nrows):
                    lo = 32 * p0 + 32
                    src = logits_sb[1:2, lo:lo + 32 * nrows].rearrange(
                        "a (p t) -> a p t", t=32)[:, :, 0:MAXLEN]
                    nc.scalar.dma_start(e_ext[p0:p0 + nrows, 32:32 + MAXLEN],
                                        src)
                if sb > 0:
                    spill(starts[sb - 1] // 32, (s0 - starts[sb - 1]) // 32)
                if sb == len(sizes) - 1:
                    spill(s0 // 32, sz // 32 - 1)

            cand = wpool.tile([128, 32 * MAXLEN], F32)
            # one fused add: cand[p, d, f] = s4[p, f] + e_ext[p, d + f]
            cand3d = cand[:].rearrange("p (d f) -> p d f", f=32)
            s4b = s4[:].unsqueeze(1).broadcast_to([128, MAXLEN, 32])
            e_base = e_ext[:]
            e_pitch = e_base.ap[0][0]
            e_win = bass.AP(e_base.tensor, e_base.offset,
                            [[e_pitch, 128], [1, MAXLEN], [1, 32]])
            nc.vector.tensor_add(cand3d, s4b, e_win)
            # special diagonal cells (1,1), (2,2), (3,3) are valid at d = 0
            nc.vector.tensor_add(cand[0:1, 1:4], logits_sb[0:1, 1:4],
                                 e_ext[0:1, 1:4])

            m8 = wpool.tile([128, 8], F32)
            i8 = wpool.tile([128, 8], U32)
            nc.vector.max(m8[:], cand[:])
            nc.vector.max_index(i8[:], m8[:], cand[:])
            nc.scalar.dma_start(out_m8[:], m8[:])
            nc.scalar.dma_start(out_i8[:], i8[:])

    nc.compile()
    return nc


def _get_nc():
    if "nc" not in _CACHE:
        _CACHE["nc"] = _build()
    return _CACHE["nc"]


def _split_bf16(a):
    """a (f32) -> (hi, lo) bf16 with hi + lo ~= a."""
    import ml_dtypes
    hi = a.astype(ml_dtypes.bfloat16)
    lo = (a - hi.astype(np.float32)).astype(ml_dtypes.bfloat16)
    return hi, lo


def run_device(seq_hiddens, ans_mask, W_qa, b_qa, trace=False, **kw):
    nc = _get_nc()
    seq_hiddens = np.asarray(seq_hiddens, dtype=np.float32)
    ans_mask = np.asarray(ans_mask, dtype=np.float32)
    w = np.asarray(W_qa, dtype=np.float32)
    wh, wl = _split_bf16(w)
    import ml_dtypes
    whl = np.zeros((H, 34), ml_dtypes.bfloat16)
    whl[:, 0:2] = wh
    whl[:, 32:34] = wl
    whl = np.ascontiguousarray(whl)
    bq = np.ascontiguousarray(np.asarray(b_qa, dtype=np.float32).reshape(2, 1))
    in_maps = []
    for b in range(N_CORES):
        xt = np.ascontiguousarray(seq_hiddens[b].T)
        xhb, xlb = _split_bf16(xt)
        in_maps.append({
            "xh": np.ascontiguousarray(xhb),
            "xl": np.ascontiguousarray(xlb),
            "whl": whl,
            "bq": bq,
            "am": np.ascontiguousarray(
                np.broadcast_to(ans_mask[b][None, :], (2, S))),
        })
    return run_bass_kernel_spmd(nc, in_maps, core_ids=list(range(N_CORES)),
                                trace=trace, **kw)


def kernel(seq_hiddens, ans_mask, W_qa, b_qa, top_k):
    k = int(top_k)
    assert k <= 8
    seq_hiddens = np.asarray(seq_hiddens, dtype=np.float32)
    ans_mask = np.asarray(ans_mask, dtype=np.float32)
    w = np.asarray(W_qa, dtype=np.float32)
    bq = np.asarray(b_qa, dtype=np.float32).reshape(2)
    res = run_device(seq_hiddens, ans_mask, w, bq)
    start_logits = np.empty((B, S), np.float32)
    end_logits = np.empty((B, S), np.float32)
    top_start = np.empty((B, k), np.int32)
    top_end = np.empty((B, k), np.int32)
    for b in range(B):
        out = res.results[b]
        start_logits[b] = out["out_logits"][0]
        end_logits[b] = out["out_logits"][1]
        # Decode the 1024 device-selected candidate spans, then re-score
        # them in exact fp32 as insurance against near-ties.
        q = out["out_i8"].astype(np.int64).ravel()            # [1024]
        p = np.arange(128, dtype=np.int64).repeat(8)
        d, f = q // 32, q % 32
        ii = 32 * p + f
        jj = ii + d
        x = seq_hiddens[b]
        m = ans_mask[b]
        s_exact = (x[ii] @ w[:, 0] + bq[0]) * m[ii] + (1.0 - m[ii]) * NEG
        e_exact = (x[jj] @ w[:, 1] + bq[1]) * m[jj] + (1.0 - m[jj]) * NEG
        score = s_exact.astype(np.float64) + e_exact.astype(np.float64)
        flat = ii * S + jj
        order = np.lexsort((flat, -score))[:k]
        top_start[b] = ii[order].astype(np.int32)
        top_end[b] = jj[order].astype(np.int32)
    return start_logits, end_logits, top_start, top_end


# revision 13
# speedup vs baseline: 1.3572x; 1.0603x over previous
"""Trainium2 Bass kernel for the QA-head top-k span-masking problem.

Computation (per batch b):
    logits = seq_hiddens[b] @ W_qa + b_qa          # (S, 2)
    masked = logits * m + (1 - m) * (-1e30)        # ans_mask
    start, end = masked[:, 0], masked[:, 1]
    span[i, j] = start[i] + end[j]  valid iff (i >= 4 and 0 <= j - i < 30)
                                     or (i == j in {1, 2, 3})
    top-5 spans by score (descending), flat index i * S + j

Sharding: pure data parallel, batch b -> NeuronCore b (B == 8 == n_cores).

seq_hiddens is staged pre-transposed and split into a bf16 hi/lo pair
(x = xh + xl, W staged as [Wh | 0 | Wl]): fp32 matmuls run at 4
cycles/row on the PE (above the DMA roofline) and float32r corrupts the
DVE max8 path on this toolchain, while bf16 runs at 1 cycle/row and the
four bf16 x bf16 cross products are exact in the fp32 PSUM accumulator,
so the split matmul matches fp32 to ~1e-6. HBM traffic is unchanged
(2 x 8.4 MB bf16 per core = the same 16.8 MB stream, the memory
roofline). The transpose is staged on the host because a 4-byte
transposed DMA degenerates to 4-byte descriptors.

Per core: the sync HWDGE ring streams x^T once (big DMAs only; the
small constant/reshape/output DMAs ride the scalar HWDGE ring so they
never stall the stream); 16 accumulating [128, 34] x [128, 512] bf16
matmuls per 512-column block -> PSUM rows 0-1 (x @ Wh) and 32-33
(x @ Wl, placed at 32 because PSUM reads must start at partition
0/32/64/96); ScalarE copies the Wl half to SBUF, VectorE folds, adds
bias, applies ans_mask into flat [2, 4096] logits; per-block DMAs
reshape start/end into a [128, 32] layout (i = 32p + f) while the
stream continues; VectorE builds the 30 shifted-diagonal candidate
bands C[p, 32d + f] = s[i] + e[i + d] in four 32-partition groups (each
scheduled as soon as its logits land) and runs the hardware
per-partition top-8 (max / max_index). The host reduces the 128x8
per-partition maxima to the global top-5, re-scoring the 1024 candidate
spans in exact fp32 as tie-safety.
"""

import numpy as np

try:
    import concourse.bass as bass  # noqa: F401
except ImportError:  # pragma: no cover - container staging path
    import sys

    sys.path.insert(0, "/opt/trn_rl_repo")

import concourse.bass as bass
import concourse.tile as tile
from concourse import bacc, mybir
from concourse.bass_utils import run_bass_kernel_spmd

B, S, H = 8, 4096, 1024
N_CORES = 8
SBLK = 512              # s-range per PSUM accumulation block
NSB = S // SBLK         # 8 s-blocks
PPB = SBLK // 32        # partition rows of the [128, 32] layout per block (16)
KC = H // 128           # 8 contraction chunks
MAXLEN = 30             # spans have 0 <= j - i < 30
NEG = -1.0e30
F32 = mybir.dt.float32
BF16 = mybir.dt.bfloat16
U32 = mybir.dt.uint32

_CACHE = {}


def _build():
    nc = bacc.Bacc("TRN2", target_bir_lowering=False, debug=False,
                   num_devices=N_CORES)
    xh = nc.dram_tensor("xh", [H, S], BF16, kind="ExternalInput").ap()
    xl = nc.dram_tensor("xl", [H, S], BF16, kind="ExternalInput").ap()
    whl = nc.dram_tensor("whl", [H, 34], BF16, kind="ExternalInput").ap()
    bq = nc.dram_tensor("bq", [2, 1], F32, kind="ExternalInput").ap()
    am = nc.dram_tensor("am", [2, S], F32, kind="ExternalInput").ap()
    nmi = nc.dram_tensor("nmi", [2, S], F32, kind="ExternalInput").ap()
    out_logits = nc.dram_tensor("out_logits", [2, S], F32,
                                kind="ExternalOutput").ap()
    out_m8 = nc.dram_tensor("out_m8", [128, 8], F32, kind="ExternalOutput").ap()
    out_i8 = nc.dram_tensor("out_i8", [128, 8], U32, kind="ExternalOutput").ap()

    with tile.TileContext(nc) as tc:
        with (
            tc.tile_pool(name="const", bufs=1) as cpool,
            tc.tile_pool(name="xin", bufs=6) as xpool,
            tc.tile_pool(name="blk", bufs=2) as bpool,
            tc.tile_pool(name="psum", bufs=3, space="PSUM") as ppool,
            tc.tile_pool(name="work", bufs=1) as wpool,
        ):
            # stationary layout [Wh(2) | zeros(30) | Wl(2)] per K-chunk, all
            # 8 chunks loaded in one DMA on the scalar ring
            w_sb = cpool.tile([128, KC, 34], BF16)
            nc.scalar.dma_start(w_sb[:],
                                whl.rearrange("(k p) c -> p k c", p=128))
            b_sb = cpool.tile([2, 1], F32)
            nc.scalar.dma_start(b_sb[:], bq[:])
            am_sb = cpool.tile([2, S], F32)
            nc.scalar.dma_start(am_sb[:], am[:])
            nm_sb = cpool.tile([2, S], F32)
            nc.scalar.dma_start(nm_sb[:], nmi[:])

            # Warm the PE HAM clock gate during the initial DMA fill: a
            # dense burst of throwaway matmuls ramps the PE to 2.4 GHz so
            # the real per-block matmul cost stays under the DMA pace.
            wu_src = cpool.tile([128, 512], BF16)
            nc.vector.memset(wu_src[:], 0.0)
            wu_ps = ppool.tile([34, 512], F32, tag="warm")
            for _ in range(16):
                nc.tensor.matmul(wu_ps[:], wu_src[:, 0:34], wu_src[:],
                                 start=True, stop=True)

            logits_sb = wpool.tile([2, S], F32)
            s4 = wpool.tile([128, 32], F32)
            e_ext = wpool.tile([128, 64], F32)
            nc.vector.memset(e_ext[96:128, 32:32 + MAXLEN], NEG)

            sizes = [512] * 7 + [256, 256]
            starts = [sum(sizes[:i]) for i in range(len(sizes))]
            for sb, (s0, sz) in enumerate(zip(starts, sizes)):
                xh_t = xpool.tile([128, KC, sz], BF16, tag="xh")
                xl_t = xpool.tile([128, KC, sz], BF16, tag="xl")
                seg = slice(s0, s0 + sz)
                nc.sync.dma_start(
                    xh_t[:], xh[:, seg].rearrange("(k p) s -> p k s", p=128))
                nc.sync.dma_start(
                    xl_t[:], xl[:, seg].rearrange("(k p) s -> p k s", p=128))
                # PSUM rows 0-1: x? @ Wh, rows 32-33: x? @ Wl; accumulating
                # both the xh and xl passes gives the exact 4-term product.
                pt = ppool.tile([34, sz], F32, tag="pt")
                for kc in range(KC):
                    nc.tensor.matmul(pt[:], w_sb[:, kc, :], xh_t[:, kc, :],
                                     start=(kc == 0), stop=False)
                for kc in range(KC):
                    nc.tensor.matmul(pt[:], w_sb[:, kc, :], xl_t[:, kc, :],
                                     start=False, stop=(kc == KC - 1))
                # fold: (pt_hi + b) + pt_lo, each op reading one PSUM operand
                t_bias = bpool.tile([2, sz], F32, tag="tbias")
                nc.vector.tensor_scalar(t_bias[:], pt[0:2, :], b_sb[:, 0:1],
                                        None, mybir.AluOpType.add)
                t_hl = bpool.tile([2, sz], F32, tag="thl")
                nc.vector.tensor_add(t_hl[:], pt[32:34, :], t_bias[:])
                # masked = (x+b)*m + nm
                t_p = bpool.tile([2, sz], F32, tag="tp")
                nc.vector.tensor_mul(t_p[:], t_hl[:], am_sb[:, seg])
                nc.vector.tensor_add(logits_sb[:, seg], t_p[:], nm_sb[:, seg])

                # Reshape this block's start/end rows into the [128, 32]
                # (i = 32p + f) layout while later blocks still stream.
                prow = slice(s0 // 32, (s0 + sz) // 32)
                last = sb == len(sizes) - 1
                ring = nc.sync if last else nc.scalar
                ring.dma_start(s4[prow, :], logits_sb[0:1, seg])
                ring.dma_start(e_ext[prow, 0:32], logits_sb[1:2, seg])
                nc.scalar.dma_start(out_logits[:, seg], logits_sb[:, seg])
                if sb == 0:
                    # start positions 0..3 are invalid for every d >= 1 and
                    # for (0,0); specials are re-added below
                    nc.vector.memset(s4[0:1, 0:4], NEG)
                # e_ext[p, 32+t] = e[32(p+1) + t] (next-partition spill) for
                # the previous block's rows: its sources end 30 elements into
                # this block. Rows 96..126 resolve within the last block; row
                # 127 keeps the NEG memset so spans with j >= S stay invalid.
                def spill(p0, nrows):
                    lo = 32 * p0 + 32
                    src = logits_sb[1:2, lo:lo + 32 * nrows].rearrange(
                        "a (p t) -> a p t", t=32)[:, :, 0:MAXLEN]
                    nc.scalar.dma_start(e_ext[p0:p0 + nrows, 32:32 + MAXLEN],
                                        src)
                if sb > 0:
                    spill(starts[sb - 1] // 32, (s0 - starts[sb - 1]) // 32)
                if sb == len(sizes) - 1:
                    spill(s0 // 32, sz // 32 - 1)

            cand = wpool.tile([128, 32 * MAXLEN], F32)
            # one fused add: cand[p, d, f] = s4[p, f] + e_ext[p, d + f]
            cand3d = cand[:].rearrange("p (d f) -> p d f", f=32)
            s4b = s4[:].unsqueeze(1).broadcast_to([128, MAXLEN, 32])
            e_base = e_ext[:]
            e_pitch = e_base.ap[0][0]
            e_win = bass.AP(e_base.tensor, e_base.offset,
                            [[e_pitch, 128], [1, MAXLEN], [1, 32]])
            nc.vector.tensor_add(cand3d, s4b, e_win)
            # special diagonal cells (1,1), (2,2), (3,3) are valid at d = 0
            nc.vector.tensor_add(cand[0:1, 1:4], logits_sb[0:1, 1:4],
                                 e_ext[0:1, 1:4])

            m8 = wpool.tile([128, 8], F32)
            i8 = wpool.tile([128, 8], U32)
            nc.vector.max(m8[:], cand[:])
            nc.vector.max_index(i8[:], m8[:], cand[:])
            nc.scalar.dma_start(out_m8[:], m8[:])
            nc.sync.dma_start(out_i8[:], i8[:])

    nc.compile()
    return nc


def _get_nc():
    if "nc" not in _CACHE:
        _CACHE["nc"] = _build()
    return _CACHE["nc"]


def _split_bf16(a):
    """a (f32) -> (hi, lo) bf16 with hi + lo ~= a."""
    import ml_dtypes
    hi = a.astype(ml_dtypes.bfloat16)
    lo = (a - hi.astype(np.float32)).astype(ml_dtypes.bfloat16)
    return hi, lo


def run_device(seq_hiddens, ans_mask, W_qa, b_qa, trace=False, **kw):
    nc = _get_nc()
    seq_hiddens = np.asarray(seq_hiddens, dtype=np.float32)
    ans_mask = np.asarray(ans_mask, dtype=np.float32)
    w = np.asarray(W_qa, dtype=np.float32)
    wh, wl = _split_bf16(w)
    import ml_dtypes
    whl = np.zeros((H, 34), ml_dtypes.bfloat16)
    whl[:, 0:2] = wh
    whl[:, 32:34] = wl
    whl = np.ascontiguousarray(whl)
    bq = np.ascontiguousarray(np.asarray(b_qa, dtype=np.float32).reshape(2, 1))
    in_maps = []
    for b in range(N_CORES):
        xt = np.ascontiguousarray(seq_hiddens[b].T)
        xhb, xlb = _split_bf16(xt)
        am2 = np.ascontiguousarray(
            np.broadcast_to(ans_mask[b][None, :], (2, S)))
        in_maps.append({
            "xh": np.ascontiguousarray(xhb),
            "xl": np.ascontiguousarray(xlb),
            "whl": whl,
            "bq": bq,
            "am": am2,
            "nmi": np.ascontiguousarray((1.0 - am2) * np.float32(NEG)),
        })
    return run_bass_kernel_spmd(nc, in_maps, core_ids=list(range(N_CORES)),
                                trace=trace, **kw)


def kernel(seq_hiddens, ans_mask, W_qa, b_qa, top_k):
    k = int(top_k)
    assert k <= 8
    seq_hiddens = np.asarray(seq_hiddens, dtype=np.float32)
    ans_mask = np.asarray(ans_mask, dtype=np.float32)
    w = np.asarray(W_qa, dtype=np.float32)
    bq = np.asarray(b_qa, dtype=np.float32).reshape(2)
    res = run_device(seq_hiddens, ans_mask, w, bq)
    start_logits = np.empty((B, S), np.float32)
    end_logits = np.empty((B, S), np.float32)
    top_start = np.empty((B, k), np.int32)
    top_end = np.empty((B, k), np.int32)
    for b in range(B):
        out = res.results[b]
        start_logits[b] = out["out_logits"][0]
        end_logits[b] = out["out_logits"][1]
        # Decode the 1024 device-selected candidate spans, then re-score
        # them in exact fp32 as insurance against near-ties.
        q = out["out_i8"].astype(np.int64).ravel()            # [1024]
        p = np.arange(128, dtype=np.int64).repeat(8)
        d, f = q // 32, q % 32
        ii = 32 * p + f
        jj = ii + d
        x = seq_hiddens[b]
        m = ans_mask[b]
        s_exact = (x[ii] @ w[:, 0] + bq[0]) * m[ii] + (1.0 - m[ii]) * NEG
        e_exact = (x[jj] @ w[:, 1] + bq[1]) * m[jj] + (1.0 - m[jj]) * NEG
        score = s_exact.astype(np.float64) + e_exact.astype(np.float64)
        flat = ii * S + jj
        order = np.lexsort((flat, -score))[:k]
        top_start[b] = ii[order].astype(np.int32)
        top_end[b] = jj[order].astype(np.int32)
    return start_logits, end_logits, top_start, top_end


# revision 14
# speedup vs baseline: 1.7764x; 1.3089x over previous
"""Trainium2 Bass kernel for the QA-head top-k span-masking problem.

Computation (per batch b):
    logits = seq_hiddens[b] @ W_qa + b_qa          # (S, 2)
    masked = logits * m + (1 - m) * (-1e30)        # ans_mask
    start, end = masked[:, 0], masked[:, 1]
    span[i, j] = start[i] + end[j]  valid iff (i >= 4 and 0 <= j - i < 30)
                                     or (i == j in {1, 2, 3})
    top-5 spans by score (descending), flat index i * S + j

Sharding: pure data parallel, batch b -> NeuronCore b (B == 8 == n_cores).

seq_hiddens is staged pre-transposed and split into a bf16 hi/lo pair
(x = xh + xl, W staged as [Wh | 0 | Wl]): fp32 matmuls run at 4
cycles/row on the PE (above the DMA roofline) and float32r corrupts the
DVE max8 path on this toolchain, while bf16 runs at 1 cycle/row and the
four bf16 x bf16 cross products are exact in the fp32 PSUM accumulator,
so the split matmul matches fp32 to ~1e-6. HBM traffic is unchanged
(2 x 8.4 MB bf16 per core = the same 16.8 MB stream, the memory
roofline). The transpose is staged on the host because a 4-byte
transposed DMA degenerates to 4-byte descriptors.

Per core: the sync HWDGE ring streams x^T once (big DMAs only; the
small constant/reshape/output DMAs ride the scalar HWDGE ring so they
never stall the stream); 16 accumulating [128, 34] x [128, 512] bf16
matmuls per 512-column block -> PSUM rows 0-1 (x @ Wh) and 32-33
(x @ Wl, placed at 32 because PSUM reads must start at partition
0/32/64/96); ScalarE copies the Wl half to SBUF, VectorE folds, adds
bias, applies ans_mask into flat [2, 4096] logits; per-block DMAs
reshape start/end into a [128, 32] layout (i = 32p + f) while the
stream continues; VectorE builds the 30 shifted-diagonal candidate
bands C[p, 32d + f] = s[i] + e[i + d] in four 32-partition groups (each
scheduled as soon as its logits land) and runs the hardware
per-partition top-8 (max / max_index). The host reduces the 128x8
per-partition maxima to the global top-5, re-scoring the 1024 candidate
spans in exact fp32 as tie-safety.
"""

import numpy as np

try:
    import concourse.bass as bass  # noqa: F401
except ImportError:  # pragma: no cover - container staging path
    import sys

    sys.path.insert(0, "/opt/trn_rl_repo")

import concourse.bass as bass
import concourse.tile as tile
from concourse import bacc, mybir
from concourse.bass_utils import run_bass_kernel_spmd

B, S, H = 8, 4096, 1024
N_CORES = 8
SBLK = 512              # s-range per PSUM accumulation block
NSB = S // SBLK         # 8 s-blocks
PPB = SBLK // 32        # partition rows of the [128, 32] layout per block (16)
KC = H // 128           # 8 contraction chunks
MAXLEN = 30             # spans have 0 <= j - i < 30
NEG = -1.0e30
F32 = mybir.dt.float32
BF16 = mybir.dt.bfloat16
U32 = mybir.dt.uint32

# Stream x as a bf16 hi/lo pair (fp32-exact logits, 16.8 MB/core) when
# True; stream only the bf16 high half (~1.5e-3 logits rel err, well
# inside the 2e-2 gate, 8.4 MB/core) when False. Top-k indices are exact
# either way via the host-side fp32 re-score of the device candidates.
EXACT_X = False

_CACHE = {}


def _build():
    nc = bacc.Bacc("TRN2", target_bir_lowering=False, debug=False,
                   num_devices=N_CORES)
    xh = nc.dram_tensor("xh", [H, S], BF16, kind="ExternalInput").ap()
    xl = (nc.dram_tensor("xl", [H, S], BF16, kind="ExternalInput").ap()
          if EXACT_X else None)
    whl = nc.dram_tensor("whl", [H, 34], BF16, kind="ExternalInput").ap()
    bq = nc.dram_tensor("bq", [2, 1], F32, kind="ExternalInput").ap()
    am = nc.dram_tensor("am", [2, S], F32, kind="ExternalInput").ap()
    nmi = nc.dram_tensor("nmi", [2, S], F32, kind="ExternalInput").ap()
    out_logits = nc.dram_tensor("out_logits", [2, S], F32,
                                kind="ExternalOutput").ap()
    out_m8 = nc.dram_tensor("out_m8", [128, 8], F32, kind="ExternalOutput").ap()
    out_i8 = nc.dram_tensor("out_i8", [128, 8], U32, kind="ExternalOutput").ap()

    with tile.TileContext(nc) as tc:
        with (
            tc.tile_pool(name="const", bufs=1) as cpool,
            tc.tile_pool(name="xin", bufs=6) as xpool,
            tc.tile_pool(name="blk", bufs=2) as bpool,
            tc.tile_pool(name="psum", bufs=3, space="PSUM") as ppool,
            tc.tile_pool(name="work", bufs=1) as wpool,
        ):
            # stationary layout [Wh(2) | zeros(30) | Wl(2)] per K-chunk, all
            # 8 chunks loaded in one DMA on the scalar ring
            w_sb = cpool.tile([128, KC, 34], BF16)
            nc.scalar.dma_start(w_sb[:],
                                whl.rearrange("(k p) c -> p k c", p=128))
            b_sb = cpool.tile([2, 1], F32)
            nc.scalar.dma_start(b_sb[:], bq[:])
            am_sb = cpool.tile([2, S], F32)
            nc.scalar.dma_start(am_sb[:], am[:])
            nm_sb = cpool.tile([2, S], F32)
            nc.scalar.dma_start(nm_sb[:], nmi[:])

            # Warm the PE HAM clock gate during the initial DMA fill: a
            # dense burst of throwaway matmuls ramps the PE to 2.4 GHz so
            # the real per-block matmul cost stays under the DMA pace.
            wu_src = cpool.tile([128, 512], BF16)
            nc.vector.memset(wu_src[:], 0.0)
            wu_ps = ppool.tile([34, 512], F32, tag="warm")
            for _ in range(16):
                nc.tensor.matmul(wu_ps[:], wu_src[:, 0:34], wu_src[:],
                                 start=True, stop=True)

            logits_sb = wpool.tile([2, S], F32)
            s4 = wpool.tile([128, 32], F32)
            e_ext = wpool.tile([128, 64], F32)
            nc.vector.memset(e_ext[96:128, 32:32 + MAXLEN], NEG)

            sizes = [512] * 7 + [256, 256]
            starts = [sum(sizes[:i]) for i in range(len(sizes))]
            for sb, (s0, sz) in enumerate(zip(starts, sizes)):
                xh_t = xpool.tile([128, KC, sz], BF16, tag="xh")
                seg = slice(s0, s0 + sz)
                nc.sync.dma_start(
                    xh_t[:], xh[:, seg].rearrange("(k p) s -> p k s", p=128))
                if EXACT_X:
                    xl_t = xpool.tile([128, KC, sz], BF16, tag="xl")
                    nc.sync.dma_start(
                        xl_t[:],
                        xl[:, seg].rearrange("(k p) s -> p k s", p=128))
                # PSUM rows 0-1: x? @ Wh, rows 32-33: x? @ Wl; accumulating
                # the xh (and optionally xl) passes sums the cross terms.
                pt = ppool.tile([34, sz], F32, tag="pt")
                for kc in range(KC):
                    nc.tensor.matmul(pt[:], w_sb[:, kc, :], xh_t[:, kc, :],
                                     start=(kc == 0),
                                     stop=(not EXACT_X and kc == KC - 1))
                if EXACT_X:
                    for kc in range(KC):
                        nc.tensor.matmul(pt[:], w_sb[:, kc, :], xl_t[:, kc, :],
                                         start=False, stop=(kc == KC - 1))
                # fold: (pt_hi + b) + pt_lo, each op reading one PSUM operand
                t_bias = bpool.tile([2, sz], F32, tag="tbias")
                nc.vector.tensor_scalar(t_bias[:], pt[0:2, :], b_sb[:, 0:1],
                                        None, mybir.AluOpType.add)
                t_hl = bpool.tile([2, sz], F32, tag="thl")
                nc.vector.tensor_add(t_hl[:], pt[32:34, :], t_bias[:])
                # masked = (x+b)*m + nm
                t_p = bpool.tile([2, sz], F32, tag="tp")
                nc.vector.tensor_mul(t_p[:], t_hl[:], am_sb[:, seg])
                nc.vector.tensor_add(logits_sb[:, seg], t_p[:], nm_sb[:, seg])

                # Reshape this block's start/end rows into the [128, 32]
                # (i = 32p + f) layout while later blocks still stream.
                prow = slice(s0 // 32, (s0 + sz) // 32)
                last = sb == len(sizes) - 1
                ring = nc.sync if last else nc.scalar
                ring.dma_start(s4[prow, :], logits_sb[0:1, seg])
                ring.dma_start(e_ext[prow, 0:32], logits_sb[1:2, seg])
                nc.scalar.dma_start(out_logits[:, seg], logits_sb[:, seg])
                if sb == 0:
                    # start positions 0..3 are invalid for every d >= 1 and
                    # for (0,0); specials are re-added below
                    nc.vector.memset(s4[0:1, 0:4], NEG)
                # e_ext[p, 32+t] = e[32(p+1) + t] (next-partition spill) for
                # the previous block's rows: its sources end 30 elements into
                # this block. Rows 96..126 resolve within the last block; row
                # 127 keeps the NEG memset so spans with j >= S stay invalid.
                def spill(p0, nrows):
                    lo = 32 * p0 + 32
                    src = logits_sb[1:2, lo:lo + 32 * nrows].rearrange(
                        "a (p t) -> a p t", t=32)[:, :, 0:MAXLEN]
                    nc.scalar.dma_start(e_ext[p0:p0 + nrows, 32:32 + MAXLEN],
                                        src)
                if sb > 0:
                    spill(starts[sb - 1] // 32, (s0 - starts[sb - 1]) // 32)
                if sb == len(sizes) - 1:
                    spill(s0 // 32, sz // 32 - 1)

            cand = wpool.tile([128, 32 * MAXLEN], F32)
            # one fused add: cand[p, d, f] = s4[p, f] + e_ext[p, d + f]
            cand3d = cand[:].rearrange("p (d f) -> p d f", f=32)
            s4b = s4[:].unsqueeze(1).broadcast_to([128, MAXLEN, 32])
            e_base = e_ext[:]
            e_pitch = e_base.ap[0][0]
            e_win = bass.AP(e_base.tensor, e_base.offset,
                            [[e_pitch, 128], [1, MAXLEN], [1, 32]])
            nc.vector.tensor_add(cand3d, s4b, e_win)
            # special diagonal cells (1,1), (2,2), (3,3) are valid at d = 0
            nc.vector.tensor_add(cand[0:1, 1:4], logits_sb[0:1, 1:4],
                                 e_ext[0:1, 1:4])

            m8 = wpool.tile([128, 8], F32)
            i8 = wpool.tile([128, 8], U32)
            nc.vector.max(m8[:], cand[:])
            nc.vector.max_index(i8[:], m8[:], cand[:])
            nc.scalar.dma_start(out_m8[:], m8[:])
            nc.sync.dma_start(out_i8[:], i8[:])

    nc.compile()
    return nc


def _get_nc():
    if "nc" not in _CACHE:
        _CACHE["nc"] = _build()
    return _CACHE["nc"]


def _split_bf16(a):
    """a (f32) -> (hi, lo) bf16 with hi + lo ~= a."""
    import ml_dtypes
    hi = a.astype(ml_dtypes.bfloat16)
    lo = (a - hi.astype(np.float32)).astype(ml_dtypes.bfloat16)
    return hi, lo


def run_device(seq_hiddens, ans_mask, W_qa, b_qa, trace=False, **kw):
    nc = _get_nc()
    seq_hiddens = np.asarray(seq_hiddens, dtype=np.float32)
    ans_mask = np.asarray(ans_mask, dtype=np.float32)
    w = np.asarray(W_qa, dtype=np.float32)
    wh, wl = _split_bf16(w)
    import ml_dtypes
    whl = np.zeros((H, 34), ml_dtypes.bfloat16)
    whl[:, 0:2] = wh
    whl[:, 32:34] = wl
    whl = np.ascontiguousarray(whl)
    bq = np.ascontiguousarray(np.asarray(b_qa, dtype=np.float32).reshape(2, 1))
    in_maps = []
    for b in range(N_CORES):
        xt = np.ascontiguousarray(seq_hiddens[b].T)
        xhb, xlb = _split_bf16(xt)
        am2 = np.ascontiguousarray(
            np.broadcast_to(ans_mask[b][None, :], (2, S)))
        m = {
            "xh": np.ascontiguousarray(xhb),
            "whl": whl,
            "bq": bq,
            "am": am2,
            "nmi": np.ascontiguousarray((1.0 - am2) * np.float32(NEG)),
        }
        if EXACT_X:
            m["xl"] = np.ascontiguousarray(xlb)
        in_maps.append(m)
    return run_bass_kernel_spmd(nc, in_maps, core_ids=list(range(N_CORES)),
                                trace=trace, **kw)


def kernel(seq_hiddens, ans_mask, W_qa, b_qa, top_k):
    k = int(top_k)
    assert k <= 8
    seq_hiddens = np.asarray(seq_hiddens, dtype=np.float32)
    ans_mask = np.asarray(ans_mask, dtype=np.float32)
    w = np.asarray(W_qa, dtype=np.float32)
    bq = np.asarray(b_qa, dtype=np.float32).reshape(2)
    res = run_device(seq_hiddens, ans_mask, w, bq)
    start_logits = np.empty((B, S), np.float32)
    end_logits = np.empty((B, S), np.float32)
    top_start = np.empty((B, k), np.int32)
    top_end = np.empty((B, k), np.int32)
    for b in range(B):
        out = res.results[b]
        start_logits[b] = out["out_logits"][0]
        end_logits[b] = out["out_logits"][1]
        # Decode the 1024 device-selected candidate spans, then re-score
        # them in exact fp32 as insurance against near-ties.
        q = out["out_i8"].astype(np.int64).ravel()            # [1024]
        p = np.arange(128, dtype=np.int64).repeat(8)
        d, f = q // 32, q % 32
        ii = 32 * p + f
        jj = ii + d
        x = seq_hiddens[b]
        m = ans_mask[b]
        s_exact = (x[ii] @ w[:, 0] + bq[0]) * m[ii] + (1.0 - m[ii]) * NEG
        e_exact = (x[jj] @ w[:, 1] + bq[1]) * m[jj] + (1.0 - m[jj]) * NEG
        score = s_exact.astype(np.float64) + e_exact.astype(np.float64)
        flat = ii * S + jj
        order = np.lexsort((flat, -score))[:k]
        top_start[b] = ii[order].astype(np.int32)
        top_end[b] = jj[order].astype(np.int32)
    return start_logits, end_logits, top_start, top_end


# revision 15
# speedup vs baseline: 1.7998x; 1.0132x over previous
"""Trainium2 Bass kernel for the QA-head top-k span-masking problem.

Computation (per batch b):
    logits = seq_hiddens[b] @ W_qa + b_qa          # (S, 2)
    masked = logits * m + (1 - m) * (-1e30)        # ans_mask
    start, end = masked[:, 0], masked[:, 1]
    span[i, j] = start[i] + end[j]  valid iff (i >= 4 and 0 <= j - i < 30)
                                     or (i == j in {1, 2, 3})
    top-5 spans by score (descending), flat index i * S + j

Sharding: pure data parallel, batch b -> NeuronCore b (B == 8 == n_cores).

seq_hiddens is staged pre-transposed and split into a bf16 hi/lo pair
(x = xh + xl, W staged as [Wh | 0 | Wl]): fp32 matmuls run at 4
cycles/row on the PE (above the DMA roofline) and float32r corrupts the
DVE max8 path on this toolchain, while bf16 runs at 1 cycle/row and the
four bf16 x bf16 cross products are exact in the fp32 PSUM accumulator,
so the split matmul matches fp32 to ~1e-6. HBM traffic is unchanged
(2 x 8.4 MB bf16 per core = the same 16.8 MB stream, the memory
roofline). The transpose is staged on the host because a 4-byte
transposed DMA degenerates to 4-byte descriptors.

Per core: the sync HWDGE ring streams x^T once (big DMAs only; the
small constant/reshape/output DMAs ride the scalar HWDGE ring so they
never stall the stream); 16 accumulating [128, 34] x [128, 512] bf16
matmuls per 512-column block -> PSUM rows 0-1 (x @ Wh) and 32-33
(x @ Wl, placed at 32 because PSUM reads must start at partition
0/32/64/96); ScalarE copies the Wl half to SBUF, VectorE folds, adds
bias, applies ans_mask into flat [2, 4096] logits; per-block DMAs
reshape start/end into a [128, 32] layout (i = 32p + f) while the
stream continues; VectorE builds the 30 shifted-diagonal candidate
bands C[p, 32d + f] = s[i] + e[i + d] in four 32-partition groups (each
scheduled as soon as its logits land) and runs the hardware
per-partition top-8 (max / max_index). The host reduces the 128x8
per-partition maxima to the global top-5, re-scoring the 1024 candidate
spans in exact fp32 as tie-safety.
"""

import numpy as np

try:
    import concourse.bass as bass  # noqa: F401
except ImportError:  # pragma: no cover - container staging path
    import sys

    sys.path.insert(0, "/opt/trn_rl_repo")

import concourse.bass as bass
import concourse.tile as tile
from concourse import bacc, mybir
from concourse.bass_utils import run_bass_kernel_spmd

B, S, H = 8, 4096, 1024
N_CORES = 8
SBLK = 512              # s-range per PSUM accumulation block
NSB = S // SBLK         # 8 s-blocks
PPB = SBLK // 32        # partition rows of the [128, 32] layout per block (16)
KC = H // 128           # 8 contraction chunks
MAXLEN = 30             # spans have 0 <= j - i < 30
NEG = -1.0e30
F32 = mybir.dt.float32
BF16 = mybir.dt.bfloat16
U32 = mybir.dt.uint32

# Stream x as a bf16 hi/lo pair (fp32-exact logits, 16.8 MB/core) when
# True; stream only the bf16 high half (~1.5e-3 logits rel err, well
# inside the 2e-2 gate, 8.4 MB/core) when False. Top-k indices are exact
# either way via the host-side fp32 re-score of the device candidates.
EXACT_X = False

_CACHE = {}


def _build():
    nc = bacc.Bacc("TRN2", target_bir_lowering=False, debug=False,
                   num_devices=N_CORES)
    xh = nc.dram_tensor("xh", [H, S], BF16, kind="ExternalInput").ap()
    xl = (nc.dram_tensor("xl", [H, S], BF16, kind="ExternalInput").ap()
          if EXACT_X else None)
    whl = nc.dram_tensor("whl", [H, 34], BF16, kind="ExternalInput").ap()
    bq = nc.dram_tensor("bq", [2, 1], F32, kind="ExternalInput").ap()
    am = nc.dram_tensor("am", [2, S], F32, kind="ExternalInput").ap()
    nmi = nc.dram_tensor("nmi", [2, S], F32, kind="ExternalInput").ap()
    out_logits = nc.dram_tensor("out_logits", [2, S], F32,
                                kind="ExternalOutput").ap()
    out_m8 = nc.dram_tensor("out_m8", [128, 8], F32, kind="ExternalOutput").ap()
    out_i8 = nc.dram_tensor("out_i8", [128, 8], U32, kind="ExternalOutput").ap()

    with tile.TileContext(nc) as tc:
        with (
            tc.tile_pool(name="const", bufs=1) as cpool,
            tc.tile_pool(name="xin", bufs=6) as xpool,
            tc.tile_pool(name="blk", bufs=2) as bpool,
            tc.tile_pool(name="psum", bufs=4, space="PSUM") as ppool,
            tc.tile_pool(name="work", bufs=1) as wpool,
        ):
            # stationary layout [Wh(2) | zeros(30) | Wl(2)] per K-chunk, all
            # 8 chunks loaded in one DMA on the scalar ring
            w_sb = cpool.tile([128, KC, 34], BF16)
            nc.scalar.dma_start(w_sb[:],
                                whl.rearrange("(k p) c -> p k c", p=128))
            b_sb = cpool.tile([2, 1], F32)
            nc.scalar.dma_start(b_sb[:], bq[:])
            am_sb = cpool.tile([2, S], F32)
            nc.scalar.dma_start(am_sb[:], am[:])
            nm_sb = cpool.tile([2, S], F32)
            nc.scalar.dma_start(nm_sb[:], nmi[:])

            # Warm the PE HAM clock gate during the initial DMA fill: a
            # dense burst of throwaway matmuls ramps the PE to 2.4 GHz so
            # the real per-block matmul cost stays under the DMA pace.
            wu_src = cpool.tile([128, 512], BF16)
            nc.vector.memset(wu_src[:], 0.0)
            wu_ps = ppool.tile([34, 512], F32, tag="warm")
            for _ in range(28):
                nc.tensor.matmul(wu_ps[:], wu_src[:, 0:34], wu_src[:],
                                 start=True, stop=True)

            logits_sb = wpool.tile([2, S], F32)
            s4 = wpool.tile([128, 32], F32)
            e_ext = wpool.tile([128, 64], F32)
            nc.vector.memset(e_ext[96:128, 32:32 + MAXLEN], NEG)

            sizes = [512] * 7 + [256, 256]
            starts = [sum(sizes[:i]) for i in range(len(sizes))]
            deferred_out = []
            for sb, (s0, sz) in enumerate(zip(starts, sizes)):
                xh_t = xpool.tile([128, KC, sz], BF16, tag="xh")
                seg = slice(s0, s0 + sz)
                nc.sync.dma_start(
                    xh_t[:], xh[:, seg].rearrange("(k p) s -> p k s", p=128))
                if EXACT_X:
                    xl_t = xpool.tile([128, KC, sz], BF16, tag="xl")
                    nc.sync.dma_start(
                        xl_t[:],
                        xl[:, seg].rearrange("(k p) s -> p k s", p=128))
                # PSUM rows 0-1: x? @ Wh, rows 32-33: x? @ Wl; accumulating
                # the xh (and optionally xl) passes sums the cross terms.
                pt = ppool.tile([34, sz], F32, tag="pt")
                for kc in range(KC):
                    nc.tensor.matmul(pt[:], w_sb[:, kc, :], xh_t[:, kc, :],
                                     start=(kc == 0),
                                     stop=(not EXACT_X and kc == KC - 1))
                if EXACT_X:
                    for kc in range(KC):
                        nc.tensor.matmul(pt[:], w_sb[:, kc, :], xl_t[:, kc, :],
                                         start=False, stop=(kc == KC - 1))
                # fold: (pt_hi + b) + pt_lo, each op reading one PSUM operand
                t_bias = bpool.tile([2, sz], F32, tag="tbias")
                nc.vector.tensor_scalar(t_bias[:], pt[0:2, :], b_sb[:, 0:1],
                                        None, mybir.AluOpType.add)
                t_hl = bpool.tile([2, sz], F32, tag="thl")
                nc.vector.tensor_add(t_hl[:], pt[32:34, :], t_bias[:])
                # masked = (x+b)*m + nm
                t_p = bpool.tile([2, sz], F32, tag="tp")
                nc.vector.tensor_mul(t_p[:], t_hl[:], am_sb[:, seg])
                nc.vector.tensor_add(logits_sb[:, seg], t_p[:], nm_sb[:, seg])

                # Reshape this block's start/end rows into the [128, 32]
                # (i = 32p + f) layout while later blocks still stream.
                prow = slice(s0 // 32, (s0 + sz) // 32)
                last = sb == len(sizes) - 1
                ring = nc.sync if last else nc.scalar
                ring.dma_start(s4[prow, :], logits_sb[0:1, seg])
                ring.dma_start(e_ext[prow, 0:32], logits_sb[1:2, seg])
                if sb < len(sizes) - 2:
                    nc.scalar.dma_start(out_logits[:, seg], logits_sb[:, seg])
                else:
                    deferred_out.append(seg)
                if sb == 0:
                    # start positions 0..3 are invalid for every d >= 1 and
                    # for (0,0); specials are re-added below
                    nc.vector.memset(s4[0:1, 0:4], NEG)
                # e_ext[p, 32+t] = e[32(p+1) + t] (next-partition spill) for
                # the previous block's rows: its sources end 30 elements into
                # this block. Rows 96..126 resolve within the last block; row
                # 127 keeps the NEG memset so spans with j >= S stay invalid.
                def spill(p0, nrows):
                    lo = 32 * p0 + 32
                    src = logits_sb[1:2, lo:lo + 32 * nrows].rearrange(
                        "a (p t) -> a p t", t=32)[:, :, 0:MAXLEN]
                    nc.scalar.dma_start(e_ext[p0:p0 + nrows, 32:32 + MAXLEN],
                                        src)
                if sb > 0:
                    spill(starts[sb - 1] // 32, (s0 - starts[sb - 1]) // 32)
                if sb == len(sizes) - 1:
                    spill(s0 // 32, sz // 32 - 1)

            cand = wpool.tile([128, 32 * MAXLEN], F32)
            # one fused add: cand[p, d, f] = s4[p, f] + e_ext[p, d + f]
            cand3d = cand[:].rearrange("p (d f) -> p d f", f=32)
            s4b = s4[:].unsqueeze(1).broadcast_to([128, MAXLEN, 32])
            e_base = e_ext[:]
            e_pitch = e_base.ap[0][0]
            e_win = bass.AP(e_base.tensor, e_base.offset,
                            [[e_pitch, 128], [1, MAXLEN], [1, 32]])
            nc.vector.tensor_add(cand3d, s4b, e_win)
            # special diagonal cells (1,1), (2,2), (3,3) are valid at d = 0
            nc.vector.tensor_add(cand[0:1, 1:4], logits_sb[0:1, 1:4],
                                 e_ext[0:1, 1:4])

            m8 = wpool.tile([128, 8], F32)
            i8 = wpool.tile([128, 8], U32)
            nc.vector.max(m8[:], cand[:])
            nc.vector.max_index(i8[:], m8[:], cand[:])
            nc.scalar.dma_start(out_m8[:], m8[:])
            nc.sync.dma_start(out_i8[:], i8[:])
            for seg in deferred_out:
                nc.scalar.dma_start(out_logits[:, seg], logits_sb[:, seg])

    nc.compile()
    return nc


def _get_nc():
    if "nc" not in _CACHE:
        _CACHE["nc"] = _build()
    return _CACHE["nc"]


def _split_bf16(a):
    """a (f32) -> (hi, lo) bf16 with hi + lo ~= a."""
    import ml_dtypes
    hi = a.astype(ml_dtypes.bfloat16)
    lo = (a - hi.astype(np.float32)).astype(ml_dtypes.bfloat16)
    return hi, lo


def run_device(seq_hiddens, ans_mask, W_qa, b_qa, trace=False, **kw):
    nc = _get_nc()
    seq_hiddens = np.asarray(seq_hiddens, dtype=np.float32)
    ans_mask = np.asarray(ans_mask, dtype=np.float32)
    w = np.asarray(W_qa, dtype=np.float32)
    wh, wl = _split_bf16(w)
    import ml_dtypes
    whl = np.zeros((H, 34), ml_dtypes.bfloat16)
    whl[:, 0:2] = wh
    whl[:, 32:34] = wl
    whl = np.ascontiguousarray(whl)
    bq = np.ascontiguousarray(np.asarray(b_qa, dtype=np.float32).reshape(2, 1))
    in_maps = []
    for b in range(N_CORES):
        xt = np.ascontiguousarray(seq_hiddens[b].T)
        xhb, xlb = _split_bf16(xt)
        am2 = np.ascontiguousarray(
            np.broadcast_to(ans_mask[b][None, :], (2, S)))
        m = {
            "xh": np.ascontiguousarray(xhb),
            "whl": whl,
            "bq": bq,
            "am": am2,
            "nmi": np.ascontiguousarray((1.0 - am2) * np.float32(NEG)),
        }
        if EXACT_X:
            m["xl"] = np.ascontiguousarray(xlb)
        in_maps.append(m)
    return run_bass_kernel_spmd(nc, in_maps, core_ids=list(range(N_CORES)),
                                trace=trace, **kw)


def kernel(seq_hiddens, ans_mask, W_qa, b_qa, top_k):
    k = int(top_k)
    assert k <= 8
    seq_hiddens = np.asarray(seq_hiddens, dtype=np.float32)
    ans_mask = np.asarray(ans_mask, dtype=np.float32)
    w = np.asarray(W_qa, dtype=np.float32)
    bq = np.asarray(b_qa, dtype=np.float32).reshape(2)
    res = run_device(seq_hiddens, ans_mask, w, bq)
    start_logits = np.empty((B, S), np.float32)
    end_logits = np.empty((B, S), np.float32)
    top_start = np.empty((B, k), np.int32)
    top_end = np.empty((B, k), np.int32)
    for b in range(B):
        out = res.results[b]
        start_logits[b] = out["out_logits"][0]
        end_logits[b] = out["out_logits"][1]
        # Decode the 1024 device-selected candidate spans, then re-score
        # them in exact fp32 as insurance against near-ties.
        q = out["out_i8"].astype(np.int64).ravel()            # [1024]
        p = np.arange(128, dtype=np.int64).repeat(8)
        d, f = q // 32, q % 32
        ii = 32 * p + f
        jj = ii + d
        x = seq_hiddens[b]
        m = ans_mask[b]
        s_exact = (x[ii] @ w[:, 0] + bq[0]) * m[ii] + (1.0 - m[ii]) * NEG
        e_exact = (x[jj] @ w[:, 1] + bq[1]) * m[jj] + (1.0 - m[jj]) * NEG
        score = s_exact.astype(np.float64) + e_exact.astype(np.float64)
        flat = ii * S + jj
        order = np.lexsort((flat, -score))[:k]
        top_start[b] = ii[order].astype(np.int32)
        top_end[b] = jj[order].astype(np.int32)
    return start_logits, end_logits, top_start, top_end
